# revision 36
# baseline (speedup 1.0000x reference)
"""Trainium2 Bass kernel for a multi-head attention layer (B=4, S=1024,
DIM=1024, H=16 heads, DH=64) with RoPE on Q/K, unmasked softmax, and output
projection.

Sharding: 8 cores = 4 batches x 2 head-halves (tensor parallel over heads).
Each core computes Q/K/V for ITS 8 heads over the full 1024-position sequence
(no duplicated projection work), attention for those heads, and the PARTIAL
output projection over its 512 o-features. The host sums the two partials per
batch and adds bo while assembling the full [B, S, DIM] output - the standard
unshard step for a row-sharded output projection, so no on-device collective
is needed.

Layouts on device (per core, all bf16 unless noted):
  xT   [128, 8, S]     x[b]^T feature-chunk major (full, shared contraction)
  wq/wk/wv [128, 8, 512]  W^T slices for own 512 out-features (in-chunk major)
  wo   [128, 4, DIM]   Wo^T rows for own 512 in-features
  csk  [128, 2, S]     cos/sin table, 2-head-stacked
  r2T  [128, 128]      transposed block-diag rotate-half matrix
  bqk  [128, 2, 4] f32 bq/bk own-half per-partition bias columns
  bvb  [128, 512]      bv own-half broadcast across partitions
  kT/qT [128, 4, S]    per head-pair chunk: rows = 2 heads' dims
  vA   [128, 8, 8, 65] v seq-major with ones column (softmax denominator)
  oT   [128, 4, S]     normalized attention output, feature-major
  outT [DIM, S]        partial output projection (no bo), transposed

All matmuls are out = lhsT.T @ rhs, cost ~ moving-dim columns. Overhead
matmuls are minimized: V bias is a vector add against bvb, the softmax
denominator rides as vA's ones column, and the reciprocal is spread across
partitions by a GpSimd partition_broadcast instead of a matmul. Schedule:
the V projection streams kc-outer right behind the input DMAs (two 4-bank
PSUM waves, x/wv chunks alternated across both HWDGE queues); K/Q projection
chunks are emitted 1.5 head-pairs ahead; the per-head logits->exp->AV chain
is pipelined depth-2 so ScalarE exp latency hides under PE work; the output
projection opens its first six accumulation groups on fc 0..2 while the last
head's normalize chain drains.
"""

import os
import numpy as np
import ml_dtypes

import concourse.bass as bass
import concourse.mybir as mybir
import concourse.tile as tile
from concourse import bacc
from concourse.bass_utils import run_bass_kernel_spmd

B, S, DIM, H, DH = 4, 1024, 1024, 16, 64
P = 128
NCORES = 8
NCH = DIM // P       # 8 chunks of 128 along the full feature dim
OCH = 4              # 4 chunks of 128 along the own 512-feature half
HOWN = 8             # heads per core
ROPE_THETA = 10000.0

BF16 = mybir.dt.bfloat16
F32 = mybir.dt.float32
AF = mybir.ActivationFunctionType

_CACHE = {}

LAST_EXEC_TIME_NS = None


def _maybe_install_trace_hook():
    """Install the NTFF profiling hook if tracing is requested (dev only)."""
    if not os.environ.get("BASS_TRACE"):
        return
    import sys, types
    if "antenv.axon_hooks" in sys.modules:
        return
    try:
        import antenv
        mod = types.ModuleType("antenv.axon_hooks")
        _state = {"hook": None}
        mod.set_axon_ntff_profile_hook = lambda h: _state.__setitem__("hook", h)
        mod.get_axon_ntff_profile_hook = lambda: _state["hook"]
        sys.modules["antenv.axon_hooks"] = mod
        antenv.axon_hooks = mod
        from trn_agent_boot.trn_boot import _ntff_profile_via_ctypes
        hook = _ntff_profile_via_ctypes("/opt/axon/libaxon_pjrt.so")
        if hook is not None:
            mod.set_axon_ntff_profile_hook(hook)
    except Exception:
        pass


def _build():
    nc = bacc.Bacc("TRN2", target_bir_lowering=False, debug=False,
                   num_devices=NCORES)

    xTd = nc.dram_tensor("xT", [DIM, S], BF16, kind="ExternalInput").ap()
    wqd = nc.dram_tensor("wq", [DIM, 512], BF16, kind="ExternalInput").ap()
    wkd = nc.dram_tensor("wk", [DIM, 512], BF16, kind="ExternalInput").ap()
    wvd = nc.dram_tensor("wv", [DIM, 512], BF16, kind="ExternalInput").ap()
    wod = nc.dram_tensor("wo", [512, DIM], BF16, kind="ExternalInput").ap()
    cskd = nc.dram_tensor("csk", [P, 2, S], BF16, kind="ExternalInput").ap()
    r2Td = nc.dram_tensor("r2T", [P, P], BF16, kind="ExternalInput").ap()
    bqkd = nc.dram_tensor("bqk", [P, 2, OCH], F32, kind="ExternalInput").ap()
    bvbd = nc.dram_tensor("bvb", [P, 512], BF16, kind="ExternalInput").ap()
    outT = nc.dram_tensor("outT", [DIM, S], BF16, kind="ExternalOutput").ap()

    with tile.TileContext(nc) as tc:
        with (
            tc.tile_pool(name="const", bufs=1) as constp,
            tc.tile_pool(name="persist", bufs=1) as pers,
            tc.tile_pool(name="zt", bufs=6) as ztp,
            tc.tile_pool(name="pT", bufs=4) as pTp,
            tc.tile_pool(name="avsb", bufs=4) as avsbp,
            tc.tile_pool(name="rcp", bufs=3) as rcpp,
            tc.tile_pool(name="bcp", bufs=3) as bcp,
            tc.tile_pool(name="outc", bufs=6) as outp,
        ):
            # ---- persistent tensors --------------------------------------
            xT_sb = pers.tile([P, NCH, S], BF16, tag="xT")
            wv_sb = pers.tile([P, NCH, 512], BF16, tag="wv")
            wk_sb = pers.tile([P, NCH, 512], BF16, tag="wk")
            wq_sb = pers.tile([P, NCH, 512], BF16, tag="wq")
            wo_sb = pers.tile([P, OCH, DIM], BF16, tag="wo")
            kT_sb = pers.tile([P, OCH, S], BF16, tag="kT")
            qT_sb = pers.tile([P, OCH, S], BF16, tag="qT")
            vA_sb = pers.tile([P, NCH, HOWN, DH + 1], BF16, tag="vA")
            oT_sb = pers.tile([P, OCH, S], BF16, tag="oT")

            nc.vector.memset(vA_sb[:, :, :, DH:DH + 1], 1.0)

            # input DMAs, 2-level contiguous patterns, in consumption order:
            # x feature-chunks interleaved with matching wv chunks so the
            # kc-outer V projection streams right behind the DMAs
            # DMA order tracks consumption: wave-1 bytes (x cols 0:512 +
            # wv + bvb) first, then wave-2 x halves, then wk (KQ0 at ~27us),
            # then rope constants, then wq and wo; alternating HWDGE queues
            nc.sync.dma_start(xT_sb[:, 0, 0:P], xTd[0:P, 0:P])
            nc.scalar.dma_start(wv_sb[:, 0, 0:256], wvd[0:P, 0:256])
            nc.sync.dma_start(wv_sb[:, 0, 256:512], wvd[0:P, 256:512])
            nc.scalar.dma_start(xT_sb[:, 0, P:S], xTd[0:P, P:S])
            for kc in range(1, NCH):
                xe = nc.sync if kc % 2 == 0 else nc.scalar
                we = nc.scalar if kc % 2 == 0 else nc.sync
                xe.dma_start(xT_sb[:, kc, :], xTd[kc * P:(kc + 1) * P, :])
                we.dma_start(wv_sb[:, kc, :], wvd[kc * P:(kc + 1) * P, :])
            bvb_sb = constp.tile([P, 512], BF16, tag="bvb")
            nc.sync.dma_start(bvb_sb[:], bvbd[:])
            for kc in range(NCH):
                we = nc.scalar if kc % 2 == 0 else nc.sync
                we.dma_start(wk_sb[:, kc, :], wkd[kc * P:(kc + 1) * P, :])
            csk_sb = constp.tile([P, 2, S], BF16, tag="csk")
            nc.sync.dma_start(csk_sb[:], cskd[:])
            r2T_sb = constp.tile([P, P], BF16, tag="r2T")
            nc.scalar.dma_start(r2T_sb[:], r2Td[:])
            bqk_sb = constp.tile([P, 2, OCH], F32, tag="bqk")
            nc.sync.dma_start(bqk_sb[:], bqkd[:])
            for kc in range(NCH):
                we = nc.scalar if kc % 2 == 0 else nc.sync
                we.dma_start(wq_sb[:, kc, :], wqd[kc * P:(kc + 1) * P, :])
            for oc in range(OCH):
                nc.scalar.dma_start(wo_sb[:, oc, :],
                                    wod[oc * P:(oc + 1) * P, :])

            # ---- V projection: out[seq-chunk 128, own-feat 512] ----------
            # single kc-outer pass over 8 accumulators (6 psv banks + 2
            # borrowed from psproj, whose first K-projection use comes
            # after the V copy-outs anyway): each chunk arrival gets its
            # full 8 matmuls of PE work, so the phase streams DMA-dense
            with tc.tile_pool(name="psproj", bufs=2, space="PSUM") as psproj:
              with tc.tile_pool(name="psv", bufs=6, space="PSUM") as psv:
                vtiles = {}
                for mt in range(NCH):
                    pool = psv if mt < 6 else psproj
                    tg = "psv" if mt < 6 else "proj"
                    vtiles[mt] = pool.tile([P, 512], F32, tag=tg,
                                           name=f"v{mt}")
                for kc in range(NCH):
                    # on the stop wave, finish the psproj-borrowed tiles
                    # (6,7) first so their copy-outs overlap the remaining
                    # stops and the K projection is not gated on them
                    mts = ([6, 7, 0, 1, 2, 3, 4, 5] if kc == NCH - 1
                           else range(NCH))
                    for mt in mts:
                        nc.tensor.matmul(
                            vtiles[mt][:],
                            xT_sb[:, kc, mt * P:(mt + 1) * P],
                            wv_sb[:, kc, :],
                            start=(kc == 0), stop=(kc == NCH - 1))
                        if kc == NCH - 1:
                            # bias added on VectorE during the PSUM->SBUF
                            # copy, interleaved so banks free incrementally
                            nc.vector.tensor_add(
                                out=vA_sb[:, mt, :, 0:DH],
                                in0=vtiles[mt].rearrange(
                                    "p (h d) -> p h d", h=HOWN),
                                in1=bvb_sb.rearrange(
                                    "p (h d) -> p h d", h=HOWN))

              with (
                tc.tile_pool(name="pslg", bufs=2, space="PSUM") as pslg,
                tc.tile_pool(name="psav", bufs=2, space="PSUM") as psav,
              ):
                # ---- K/Q projection + RoPE for head-pair chunk mt --------
                def proj_rope(out_sb, mt, w_sb, bcol):
                    for ns in range(2):  # seq halves, N=512 each
                        ps = psproj.tile([P, 512], F32, tag="proj", name="kqps")
                        for kc in range(NCH):
                            nc.tensor.matmul(
                                ps[:], w_sb[:, kc, mt * P:(mt + 1) * P],
                                xT_sb[:, kc, ns * 512:(ns + 1) * 512],
                                start=(kc == 0), stop=(kc == NCH - 1))
                        zsb = ztp.tile([P, 512], BF16, tag="zt", name="zsb")
                        nc.scalar.activation(zsb[:], ps[:], AF.Identity,
                                             bias=bqk_sb[:, bcol, mt:mt + 1])
                        rpool, rtag = ((psav, "av") if mt < 2
                                       else (psproj, "proj"))
                        rot = rpool.tile([P, 512], F32, tag=rtag, name="rot")
                        nc.tensor.matmul(rot[:], r2T_sb[:], zsb[:],
                                         start=True, stop=True)
                        t1 = ztp.tile([P, 512], BF16, tag="zt", name="t1")
                        nc.vector.tensor_mul(
                            out=t1[:], in0=zsb[:],
                            in1=csk_sb[:, 0, ns * 512:(ns + 1) * 512])
                        t2 = ztp.tile([P, 512], BF16, tag="zt", name="t2")
                        nc.vector.tensor_mul(
                            out=t2[:], in0=rot[:],
                            in1=csk_sb[:, 1, ns * 512:(ns + 1) * 512])
                        nc.vector.tensor_add(
                            out=out_sb[:, mt, ns * 512:(ns + 1) * 512],
                            in0=t1[:], in1=t2[:])

                def emit_k(mt):
                    proj_rope(kT_sb, mt, wk_sb, 1)

                def emit_q(mt):
                    proj_rope(qT_sb, mt, wq_sb, 0)

                # ---- attention per own head h (pair mt=h//2, poff=(h%2)*64) --
                def emit_logits(h):
                    """logits + exp -> pT[h]; returns the pT tile."""
                    mt, poff = h // 2, (h % 2) * DH
                    pt = pTp.tile([P, NCH, 2, 512], BF16, tag="pT", name="pt")
                    for kt in range(NCH):
                        lg = pslg.tile([P, 2, 512], F32, tag="lg", name="lg")
                        for j in range(2):  # q halves, N=512
                            nc.tensor.matmul(
                                lg[:, j, :],
                                kT_sb[poff:poff + DH, mt, kt * P:(kt + 1) * P],
                                qT_sb[poff:poff + DH, mt, j * 512:(j + 1) * 512],
                                start=True, stop=True)
                        nc.scalar.activation(pt[:, kt, :, :], lg[:],
                                             AF.Exp, scale=0.125)
                    return pt

                def emit_av(h, pt):
                    mt, hip = h // 2, h % 2
                    poff = hip * DH
                    for j in range(2):  # q halves
                        av = psav.tile([P, 512], F32, tag="av",
                                       name="av")[:DH + 1, :]
                        for kt in range(NCH):
                            nc.tensor.matmul(
                                av, vA_sb[:, kt, h, :],
                                pt[:, kt, j, :],
                                start=(kt == 0), stop=(kt == NCH - 1))
                        # stage numerator+denominator out of PSUM fast so
                        # the next accumulation gets its bank back (and the
                        # output projection's early-open groups, which reuse
                        # this PSUM zone, are not gated on the normalize
                        # chain); the bit-trick reciprocal cannot read PSUM
                        avsb = avsbp.tile([P, 512], F32, tag="avsb",
                                          name="avsb")
                        nc.vector.tensor_copy(
                            out=avsb[poff:poff + DH, :], in_=av[0:DH, :])
                        den = rcpp.tile([1, 512], F32, tag="rcp", name="den")
                        nc.vector.tensor_copy(out=den[:],
                                              in_=av[DH:DH + 1, :])
                        rtmp = rcpp.tile([1, 512], F32, tag="rcp",
                                         name="rtmp")
                        nc.vector.reciprocal_approx_fast(out=rtmp[:],
                                                         in_=den[:])
                        bc = bcp.tile([P, 512], F32, tag="bc", name="bc")
                        nc.gpsimd.partition_broadcast(bc[:], rtmp[:],
                                                      channels=P)
                        nc.vector.tensor_mul(
                            out=oT_sb[poff:poff + DH, mt,
                                      j * 512:(j + 1) * 512],
                            in0=avsb[poff:poff + DH, :],
                            in1=bc[poff:poff + DH, :])

                def oproj_matmul(ps, ec, j, fc):
                    nc.tensor.matmul(
                        ps[:], wo_sb[:, fc, ec * P:(ec + 1) * P],
                        oT_sb[:, fc, j * 512:(j + 1) * 512],
                        start=(fc == 0), stop=(fc == OCH - 1))

                def opool(i):
                    k = i % 6
                    if k < 2:
                        return psproj.tile([P, 512], F32, tag="proj",
                                           name="ops")
                    if k < 4:
                        return pslg.tile([P, 2, 512], F32, tag="lg",
                                         name="olg")[:, 0, :]
                    return psav.tile([P, 512], F32, tag="av", name="oav")

                gi = 0
                osbs = {}
                earlies = []

                def open_early(ec):
                    nonlocal gi
                    osbs[ec] = outp.tile([P, S], BF16, tag="outc",
                                         name="osb")
                    for j in range(2):
                        ps = opool(gi)
                        gi += 1
                        for fc in range(OCH - 1):
                            oproj_matmul(ps, ec, j, fc)
                        earlies.append((ec, j, ps))

                # pipeline: one K- or Q-projection half-pair per head
                # slot (smoother PE load, and each rope chain's vector tail
                # is covered by the next slot's matmuls); logits run a head
                # ahead of AV; ec=0's output-projection groups open right
                # after AV(6) to fill the last exp's latency
                emit_k(0)
                emit_q(0)
                emit_k(1)
                pts = {0: emit_logits(0)}
                kq_sched = {0: lambda: emit_q(1), 1: lambda: emit_k(2),
                            2: lambda: emit_q(2), 3: lambda: emit_k(3),
                            4: lambda: emit_q(3)}
                for h in range(HOWN):
                    if h in kq_sched:
                        kq_sched[h]()
                    if h + 1 < HOWN:
                        pts[h + 1] = emit_logits(h + 1)
                    emit_av(h, pts.pop(h))
                    if h == HOWN - 2:
                        open_early(0)

                # ---- partial output projection (no bias; host adds bo) -----
                # accumulators are drawn from the three still-open attention
                # pools instead of a fresh pool: a fresh pool's PSUM zone would
                # be gated on the LAST reader of every prior pool (the final
                # normalize staging copies), stalling the PE ~3us. psproj's
                # buffers have been quiet longest, so the first early-open
                # groups start right behind the last AV matmuls.
                for ec in range(1, 3):
                    open_early(ec)
                for ec, j, ps in sorted(earlies,
                                        key=lambda t: (t[1], t[0])):
                    oproj_matmul(ps, ec, j, OCH - 1)
                    nc.scalar.activation(
                        osbs[ec][:, j * 512:(j + 1) * 512], ps[:], AF.Identity)
                    if j == 1:
                        nc.sync.dma_start(outT[ec * P:(ec + 1) * P, :],
                                          osbs[ec])
                for ec in range(3, NCH):
                    osb = outp.tile([P, S], BF16, tag="outc", name="osb")
                    for j in range(2):
                        ps = opool(gi)
                        gi += 1
                        for fc in range(OCH):
                            oproj_matmul(ps, ec, j, fc)
                        nc.scalar.activation(
                            osb[:, j * 512:(j + 1) * 512], ps[:], AF.Identity)
                        if ec == NCH - 1:
                            nc.sync.dma_start(
                                outT[ec * P:(ec + 1) * P,
                                     j * 512:(j + 1) * 512],
                                osb[:, j * 512:(j + 1) * 512])
                    if ec < NCH - 1:
                        nc.sync.dma_start(outT[ec * P:(ec + 1) * P, :], osb[:])

    nc.compile()
    return nc


def _host_tables():
    half = DH // 2
    freqs = 1.0 / (ROPE_THETA ** (np.arange(0, DH, 2, dtype=np.float64)[:half]
                                  / DH))
    ang = np.outer(np.arange(S, dtype=np.float64), freqs)      # (S, 32)
    cos64 = np.tile(np.cos(ang), (1, 2)).T.astype(np.float32)  # (64, S)
    sin64 = np.tile(np.sin(ang), (1, 2)).T.astype(np.float32)
    cos128 = np.concatenate([cos64, cos64], 0)
    sin128 = np.concatenate([sin64, sin64], 0)
    csk = np.ascontiguousarray(np.stack([cos128, sin128], 1))  # (128, 2, S)

    R64 = np.zeros((DH, DH), np.float32)
    for d in range(half):
        R64[d, d + half] = -1.0
        R64[d + half, d] = 1.0
    R2 = np.zeros((P, P), np.float32)
    R2[:DH, :DH] = R64
    R2[DH:, DH:] = R64

    return csk, np.ascontiguousarray(R2.T)


def kernel(x, Wq, bq, Wk, bk, Wv, bv, Wo, bo):
    global LAST_EXEC_TIME_NS
    _maybe_install_trace_hook()
    bf = ml_dtypes.bfloat16

    if "nc" not in _CACHE:
        _CACHE["nc"] = _build()
        _CACHE["tables"] = _host_tables()
    nc = _CACHE["nc"]
    csk, r2T = _CACHE["tables"]

    x = np.asarray(x, np.float32)
    Wq = np.asarray(Wq, np.float32)
    Wk = np.asarray(Wk, np.float32)
    Wv = np.asarray(Wv, np.float32)
    Wo = np.asarray(Wo, np.float32)
    bq = np.asarray(bq, np.float32)
    bk = np.asarray(bk, np.float32)
    bv = np.asarray(bv, np.float32)
    bo = np.asarray(bo, np.float32)

    in_maps = []
    for c in range(NCORES):
        b, hh = c // 2, c % 2
        own = slice(hh * 512, (hh + 1) * 512)
        bqk = np.ascontiguousarray(
            np.stack([bq[own].reshape(OCH, P).T,
                      bk[own].reshape(OCH, P).T], 1))        # [128, 2, 4]
        in_maps.append({
            "xT": np.ascontiguousarray(x[b].T).astype(bf),
            "wq": np.ascontiguousarray(Wq[own, :].T).astype(bf),
            "wk": np.ascontiguousarray(Wk[own, :].T).astype(bf),
            "wv": np.ascontiguousarray(Wv[own, :].T).astype(bf),
            "wo": np.ascontiguousarray(Wo[:, own].T).astype(bf),
            "csk": csk.astype(bf),
            "r2T": r2T.astype(bf),
            "bqk": bqk,
            "bvb": np.ascontiguousarray(
                np.broadcast_to(bv[own], (P, 512))).astype(bf),
        })

    res = run_bass_kernel_spmd(nc, in_maps, list(range(NCORES)))
    LAST_EXEC_TIME_NS = res.exec_time_ns

    out = np.empty((B, S, DIM), np.float32)
    for b in range(B):
        pa = res.results[2 * b]["outT"].astype(np.float32)
        pb = res.results[2 * b + 1]["outT"].astype(np.float32)
        out[b] = (pa + pb).T + bo
    return out



# revision 37
# speedup vs baseline: 1.6087x; 1.6087x over previous
"""Trainium2 Bass kernel for a multi-head attention layer (B=4, S=1024,
DIM=1024, H=16 heads, DH=64) with RoPE on Q/K, unmasked softmax, and output
projection.

Sharding: 8 cores = 4 batches x 2 head-halves (tensor parallel over heads).
Each core computes Q/K/V for ITS 8 heads over the full 1024-position sequence
(no duplicated projection work), attention for those heads, and the PARTIAL
output projection over its 512 o-features. The host sums the two partials per
batch and adds bo while assembling the full [B, S, DIM] output - the standard
unshard step for a row-sharded output projection, so no on-device collective
is needed.

Layouts on device (per core, all bf16 unless noted):
  xT   [128, 8, S]     x[b]^T feature-chunk major (full, shared contraction)
  wq/wk/wv [128, 8, 512]  W^T slices for own 512 out-features (in-chunk major)
  wo   [128, 4, DIM]   Wo^T rows for own 512 in-features
  csk  [128, 2, S]     cos/sin table, 2-head-stacked
  r2T  [128, 128]      transposed block-diag rotate-half matrix
  bqk  [128, 2, 4] f32 bq/bk own-half per-partition bias columns
  bvb  [128, 512]      bv own-half broadcast across partitions
  kT/qT [128, 4, S]    per head-pair chunk: rows = 2 heads' dims
  vA   [128, 8, 8, 65] v seq-major with ones column (softmax denominator)
  oT   [128, 4, S]     normalized attention output, feature-major
  outT [DIM, S]        partial output projection (no bo), transposed

All matmuls are out = lhsT.T @ rhs, cost ~ moving-dim columns. Overhead
matmuls are minimized: V bias is a vector add against bvb, the softmax
denominator rides as vA's ones column, and the reciprocal is spread across
partitions by a GpSimd partition_broadcast instead of a matmul. Schedule:
the V projection streams kc-outer right behind the input DMAs (two 4-bank
PSUM waves, x/wv chunks alternated across both HWDGE queues); K/Q projection
chunks are emitted 1.5 head-pairs ahead; the per-head logits->exp->AV chain
is pipelined depth-2 so ScalarE exp latency hides under PE work; the output
projection opens its first six accumulation groups on fc 0..2 while the last
head's normalize chain drains.
"""

import os
import numpy as np
import ml_dtypes

import concourse.bass as bass
import concourse.mybir as mybir
import concourse.tile as tile
from concourse import bacc
from concourse.bass_utils import run_bass_kernel_spmd

B, S, DIM, H, DH = 4, 1024, 1024, 16, 64
P = 128
NCORES = 8
NCH = DIM // P       # 8 chunks of 128 along the full feature dim
OCH = 4              # 4 chunks of 128 along the own 512-feature half
HOWN = 8             # heads per core
ROPE_THETA = 10000.0

BF16 = mybir.dt.bfloat16
F32 = mybir.dt.float32
AF = mybir.ActivationFunctionType

_CACHE = {}

LAST_EXEC_TIME_NS = None


def _maybe_install_trace_hook():
    """Install the NTFF profiling hook if tracing is requested (dev only)."""
    if not os.environ.get("BASS_TRACE"):
        return
    import sys, types
    if "antenv.axon_hooks" in sys.modules:
        return
    try:
        import antenv
        mod = types.ModuleType("antenv.axon_hooks")
        _state = {"hook": None}
        mod.set_axon_ntff_profile_hook = lambda h: _state.__setitem__("hook", h)
        mod.get_axon_ntff_profile_hook = lambda: _state["hook"]
        sys.modules["antenv.axon_hooks"] = mod
        antenv.axon_hooks = mod
        from trn_agent_boot.trn_boot import _ntff_profile_via_ctypes
        hook = _ntff_profile_via_ctypes("/opt/axon/libaxon_pjrt.so")
        if hook is not None:
            mod.set_axon_ntff_profile_hook(hook)
    except Exception:
        pass


def _build():
    nc = bacc.Bacc("TRN2", target_bir_lowering=False, debug=False,
                   num_devices=NCORES)

    xTd = nc.dram_tensor("xT", [DIM, S], BF16, kind="ExternalInput").ap()
    wqd = nc.dram_tensor("wq", [DIM, 512], BF16, kind="ExternalInput").ap()
    wkd = nc.dram_tensor("wk", [DIM, 512], BF16, kind="ExternalInput").ap()
    wvd = nc.dram_tensor("wv", [DIM, 512], BF16, kind="ExternalInput").ap()
    wod = nc.dram_tensor("wo", [512, DIM], BF16, kind="ExternalInput").ap()
    cskd = nc.dram_tensor("csk", [P, 2, S], BF16, kind="ExternalInput").ap()
    r2Td = nc.dram_tensor("r2T", [P, P], BF16, kind="ExternalInput").ap()
    bqkd = nc.dram_tensor("bqk", [P, 2, OCH], F32, kind="ExternalInput").ap()
    bvbd = nc.dram_tensor("bvb", [P, 512], BF16, kind="ExternalInput").ap()
    outT = nc.dram_tensor("outT", [DIM, S], BF16, kind="ExternalOutput").ap()

    with tile.TileContext(nc) as tc:
        with (
            tc.tile_pool(name="const", bufs=1) as constp,
            tc.tile_pool(name="persist", bufs=1) as pers,
            tc.tile_pool(name="zt", bufs=6) as ztp,
            tc.tile_pool(name="pT", bufs=4) as pTp,
            tc.tile_pool(name="avsb", bufs=4) as avsbp,
            tc.tile_pool(name="rcp", bufs=3) as rcpp,
            tc.tile_pool(name="bcp", bufs=3) as bcp,
            tc.tile_pool(name="outc", bufs=6) as outp,
        ):
            # ---- persistent tensors --------------------------------------
            xT_sb = pers.tile([P, NCH, S], BF16, tag="xT")
            wv_sb = pers.tile([P, NCH, 512], BF16, tag="wv")
            wk_sb = pers.tile([P, NCH, 512], BF16, tag="wk")
            wq_sb = pers.tile([P, NCH, 512], BF16, tag="wq")
            wo_sb = pers.tile([P, OCH, DIM], BF16, tag="wo")
            kT_sb = pers.tile([P, OCH, S], BF16, tag="kT")
            qT_sb = pers.tile([P, OCH, S], BF16, tag="qT")
            vA_sb = pers.tile([P, NCH, HOWN, DH + 1], BF16, tag="vA")
            oT_sb = pers.tile([P, OCH, S], BF16, tag="oT")

            nc.vector.memset(vA_sb[:, :, :, DH:DH + 1], 1.0)

            # PE warmup: tiny matmuls on a zeroed scratch tile keep the HAM
            # activity monitor busy during the input-DMA wait so the first
            # real matmuls run at the full 2.4 GHz clock
            warm_sb = constp.tile([P, P], BF16, tag="warm")
            nc.vector.memset(warm_sb[:], 0.0)

            # input DMAs, 2-level contiguous patterns, in consumption order:
            # x feature-chunks interleaved with matching wv chunks so the
            # kc-outer V projection streams right behind the DMAs
            # DMA order tracks consumption: wave-1 bytes (x cols 0:512 +
            # wv + bvb) first, then wave-2 x halves, then wk (KQ0 at ~27us),
            # then rope constants, then wq and wo; alternating HWDGE queues
            nc.sync.dma_start(xT_sb[:, 0, 0:P], xTd[0:P, 0:P])
            nc.scalar.dma_start(wv_sb[:, 0, 0:256], wvd[0:P, 0:256])
            nc.sync.dma_start(wv_sb[:, 0, 256:512], wvd[0:P, 256:512])
            nc.scalar.dma_start(xT_sb[:, 0, P:S], xTd[0:P, P:S])
            for kc in range(1, NCH):
                xe = nc.sync if kc % 2 == 0 else nc.scalar
                we = nc.scalar if kc % 2 == 0 else nc.sync
                xe.dma_start(xT_sb[:, kc, :], xTd[kc * P:(kc + 1) * P, :])
                we.dma_start(wv_sb[:, kc, :], wvd[kc * P:(kc + 1) * P, :])
            bvb_sb = constp.tile([P, 512], BF16, tag="bvb")
            nc.sync.dma_start(bvb_sb[:], bvbd[:])
            for kc in range(NCH):
                we = nc.scalar if kc % 2 == 0 else nc.sync
                we.dma_start(wk_sb[:, kc, :], wkd[kc * P:(kc + 1) * P, :])
            csk_sb = constp.tile([P, 2, S], BF16, tag="csk")
            nc.sync.dma_start(csk_sb[:], cskd[:])
            r2T_sb = constp.tile([P, P], BF16, tag="r2T")
            nc.scalar.dma_start(r2T_sb[:], r2Td[:])
            bqk_sb = constp.tile([P, 2, OCH], F32, tag="bqk")
            nc.sync.dma_start(bqk_sb[:], bqkd[:])
            for kc in range(NCH):
                we = nc.scalar if kc % 2 == 0 else nc.sync
                we.dma_start(wq_sb[:, kc, :], wqd[kc * P:(kc + 1) * P, :])
            for oc in range(OCH):
                nc.scalar.dma_start(wo_sb[:, oc, :],
                                    wod[oc * P:(oc + 1) * P, :])

            # ---- V projection: out[seq-chunk 128, own-feat 512] ----------
            # single kc-outer pass over 8 accumulators (6 psv banks + 2
            # borrowed from psproj, whose first K-projection use comes
            # after the V copy-outs anyway): each chunk arrival gets its
            # full 8 matmuls of PE work, so the phase streams DMA-dense
            with tc.tile_pool(name="psproj", bufs=2, space="PSUM") as psproj:
              with tc.tile_pool(name="psv", bufs=6, space="PSUM") as psv:
                warm_ps = psv.tile([P, P], F32, tag="psv", name="warm")
                for _ in range(32):
                    nc.tensor.matmul(warm_ps[:], warm_sb[:], warm_sb[:],
                                     start=True, stop=True)
                vtiles = {}
                for mt in range(NCH):
                    pool = psv if mt < 6 else psproj
                    tg = "psv" if mt < 6 else "proj"
                    vtiles[mt] = pool.tile([P, 512], F32, tag=tg,
                                           name=f"v{mt}")
                for kc in range(NCH):
                    # on the stop wave, finish the psproj-borrowed tiles
                    # (6,7) first so their copy-outs overlap the remaining
                    # stops and the K projection is not gated on them
                    mts = ([6, 7, 0, 1, 2, 3, 4, 5] if kc == NCH - 1
                           else range(NCH))
                    for mt in mts:
                        nc.tensor.matmul(
                            vtiles[mt][:],
                            xT_sb[:, kc, mt * P:(mt + 1) * P],
                            wv_sb[:, kc, :],
                            start=(kc == 0), stop=(kc == NCH - 1))
                        if kc == NCH - 1:
                            # bias added on VectorE during the PSUM->SBUF
                            # copy, interleaved so banks free incrementally
                            nc.vector.tensor_add(
                                out=vA_sb[:, mt, :, 0:DH],
                                in0=vtiles[mt].rearrange(
                                    "p (h d) -> p h d", h=HOWN),
                                in1=bvb_sb.rearrange(
                                    "p (h d) -> p h d", h=HOWN))

              with (
                tc.tile_pool(name="pslg", bufs=2, space="PSUM") as pslg,
                tc.tile_pool(name="psav", bufs=2, space="PSUM") as psav,
              ):
                # ---- K/Q projection + RoPE for head-pair chunk mt --------
                def proj_rope(out_sb, mt, w_sb, bcol):
                    for ns in range(2):  # seq halves, N=512 each
                        ps = psproj.tile([P, 512], F32, tag="proj", name="kqps")
                        for kc in range(NCH):
                            nc.tensor.matmul(
                                ps[:], w_sb[:, kc, mt * P:(mt + 1) * P],
                                xT_sb[:, kc, ns * 512:(ns + 1) * 512],
                                start=(kc == 0), stop=(kc == NCH - 1))
                        zsb = ztp.tile([P, 512], BF16, tag="zt", name="zsb")
                        nc.scalar.activation(zsb[:], ps[:], AF.Identity,
                                             bias=bqk_sb[:, bcol, mt:mt + 1])
                        rpool, rtag = ((psav, "av") if mt < 2
                                       else (psproj, "proj"))
                        rot = rpool.tile([P, 512], F32, tag=rtag, name="rot")
                        nc.tensor.matmul(rot[:], r2T_sb[:], zsb[:],
                                         start=True, stop=True)
                        t1 = ztp.tile([P, 512], BF16, tag="zt", name="t1")
                        nc.vector.tensor_mul(
                            out=t1[:], in0=zsb[:],
                            in1=csk_sb[:, 0, ns * 512:(ns + 1) * 512])
                        t2 = ztp.tile([P, 512], BF16, tag="zt", name="t2")
                        nc.vector.tensor_mul(
                            out=t2[:], in0=rot[:],
                            in1=csk_sb[:, 1, ns * 512:(ns + 1) * 512])
                        nc.vector.tensor_add(
                            out=out_sb[:, mt, ns * 512:(ns + 1) * 512],
                            in0=t1[:], in1=t2[:])

                def emit_k(mt):
                    proj_rope(kT_sb, mt, wk_sb, 1)

                def emit_q(mt):
                    proj_rope(qT_sb, mt, wq_sb, 0)

                # ---- attention per own head h (pair mt=h//2, poff=(h%2)*64) --
                def emit_logits(h):
                    """logits + exp -> pT[h]; returns the pT tile."""
                    mt, poff = h // 2, (h % 2) * DH
                    pt = pTp.tile([P, NCH, 2, 512], BF16, tag="pT", name="pt")
                    for kt in range(NCH):
                        lg = pslg.tile([P, 2, 512], F32, tag="lg", name="lg")
                        for j in range(2):  # q halves, N=512
                            nc.tensor.matmul(
                                lg[:, j, :],
                                kT_sb[poff:poff + DH, mt, kt * P:(kt + 1) * P],
                                qT_sb[poff:poff + DH, mt, j * 512:(j + 1) * 512],
                                start=True, stop=True)
                        nc.scalar.activation(pt[:, kt, :, :], lg[:],
                                             AF.Exp, scale=0.125)
                    return pt

                def emit_av(h, pt):
                    mt, hip = h // 2, h % 2
                    poff = hip * DH
                    for j in range(2):  # q halves
                        av = psav.tile([P, 512], F32, tag="av",
                                       name="av")[:DH + 1, :]
                        for kt in range(NCH):
                            nc.tensor.matmul(
                                av, vA_sb[:, kt, h, :],
                                pt[:, kt, j, :],
                                start=(kt == 0), stop=(kt == NCH - 1))
                        # stage numerator+denominator out of PSUM fast so
                        # the next accumulation gets its bank back (and the
                        # output projection's early-open groups, which reuse
                        # this PSUM zone, are not gated on the normalize
                        # chain); the bit-trick reciprocal cannot read PSUM
                        avsb = avsbp.tile([P, 512], F32, tag="avsb",
                                          name="avsb")
                        nc.vector.tensor_copy(
                            out=avsb[poff:poff + DH, :], in_=av[0:DH, :])
                        den = rcpp.tile([1, 512], F32, tag="rcp", name="den")
                        nc.vector.tensor_copy(out=den[:],
                                              in_=av[DH:DH + 1, :])
                        rtmp = rcpp.tile([1, 512], F32, tag="rcp",
                                         name="rtmp")
                        nc.vector.reciprocal_approx_fast(out=rtmp[:],
                                                         in_=den[:])
                        bc = bcp.tile([P, 512], F32, tag="bc", name="bc")
                        nc.gpsimd.partition_broadcast(bc[:], rtmp[:],
                                                      channels=P)
                        nc.vector.tensor_mul(
                            out=oT_sb[poff:poff + DH, mt,
                                      j * 512:(j + 1) * 512],
                            in0=avsb[poff:poff + DH, :],
                            in1=bc[poff:poff + DH, :])

                def oproj_matmul(ps, ec, j, fc):
                    nc.tensor.matmul(
                        ps[:], wo_sb[:, fc, ec * P:(ec + 1) * P],
                        oT_sb[:, fc, j * 512:(j + 1) * 512],
                        start=(fc == 0), stop=(fc == OCH - 1))

                def opool(i):
                    k = i % 6
                    if k < 2:
                        return psproj.tile([P, 512], F32, tag="proj",
                                           name="ops")
                    if k < 4:
                        return pslg.tile([P, 2, 512], F32, tag="lg",
                                         name="olg")[:, 0, :]
                    return psav.tile([P, 512], F32, tag="av", name="oav")

                gi = 0
                osbs = {}
                earlies = []

                def open_early(ec):
                    nonlocal gi
                    osbs[ec] = outp.tile([P, S], BF16, tag="outc",
                                         name="osb")
                    for j in range(2):
                        ps = opool(gi)
                        gi += 1
                        for fc in range(OCH - 1):
                            oproj_matmul(ps, ec, j, fc)
                        earlies.append((ec, j, ps))

                # pipeline: one K- or Q-projection half-pair per head
                # slot (smoother PE load, and each rope chain's vector tail
                # is covered by the next slot's matmuls); logits run a head
                # ahead of AV; ec=0's output-projection groups open right
                # after AV(6) to fill the last exp's latency
                emit_k(0)
                emit_q(0)
                emit_k(1)
                pts = {0: emit_logits(0)}
                kq_sched = {0: lambda: emit_q(1), 1: lambda: emit_k(2),
                            2: lambda: emit_q(2), 3: lambda: emit_k(3),
                            4: lambda: emit_q(3)}
                for h in range(HOWN):
                    if h in kq_sched:
                        kq_sched[h]()
                    if h + 1 < HOWN:
                        pts[h + 1] = emit_logits(h + 1)
                    emit_av(h, pts.pop(h))
                    if h == HOWN - 2:
                        open_early(0)

                # ---- partial output projection (no bias; host adds bo) -----
                # accumulators are drawn from the three still-open attention
                # pools instead of a fresh pool: a fresh pool's PSUM zone would
                # be gated on the LAST reader of every prior pool (the final
                # normalize staging copies), stalling the PE ~3us. psproj's
                # buffers have been quiet longest, so the first early-open
                # groups start right behind the last AV matmuls.
                for ec in range(1, 3):
                    open_early(ec)
                for ec, j, ps in sorted(earlies,
                                        key=lambda t: (t[1], t[0])):
                    oproj_matmul(ps, ec, j, OCH - 1)
                    nc.scalar.activation(
                        osbs[ec][:, j * 512:(j + 1) * 512], ps[:], AF.Identity)
                    if j == 1:
                        nc.sync.dma_start(outT[ec * P:(ec + 1) * P, :],
                                          osbs[ec])
                for ec in range(3, NCH):
                    osb = outp.tile([P, S], BF16, tag="outc", name="osb")
                    for j in range(2):
                        ps = opool(gi)
                        gi += 1
                        for fc in range(OCH):
                            oproj_matmul(ps, ec, j, fc)
                        nc.scalar.activation(
                            osb[:, j * 512:(j + 1) * 512], ps[:], AF.Identity)
                        if ec == NCH - 1:
                            nc.sync.dma_start(
                                outT[ec * P:(ec + 1) * P,
                                     j * 512:(j + 1) * 512],
                                osb[:, j * 512:(j + 1) * 512])
                    if ec < NCH - 1:
                        nc.sync.dma_start(outT[ec * P:(ec + 1) * P, :], osb[:])

    nc.compile()
    return nc


def _host_tables():
    half = DH // 2
    freqs = 1.0 / (ROPE_THETA ** (np.arange(0, DH, 2, dtype=np.float64)[:half]
                                  / DH))
    ang = np.outer(np.arange(S, dtype=np.float64), freqs)      # (S, 32)
    cos64 = np.tile(np.cos(ang), (1, 2)).T.astype(np.float32)  # (64, S)
    sin64 = np.tile(np.sin(ang), (1, 2)).T.astype(np.float32)
    cos128 = np.concatenate([cos64, cos64], 0)
    sin128 = np.concatenate([sin64, sin64], 0)
    csk = np.ascontiguousarray(np.stack([cos128, sin128], 1))  # (128, 2, S)

    R64 = np.zeros((DH, DH), np.float32)
    for d in range(half):
        R64[d, d + half] = -1.0
        R64[d + half, d] = 1.0
    R2 = np.zeros((P, P), np.float32)
    R2[:DH, :DH] = R64
    R2[DH:, DH:] = R64

    return csk, np.ascontiguousarray(R2.T)


def kernel(x, Wq, bq, Wk, bk, Wv, bv, Wo, bo):
    global LAST_EXEC_TIME_NS
    _maybe_install_trace_hook()
    bf = ml_dtypes.bfloat16

    if "nc" not in _CACHE:
        _CACHE["nc"] = _build()
        _CACHE["tables"] = _host_tables()
    nc = _CACHE["nc"]
    csk, r2T = _CACHE["tables"]

    x = np.asarray(x, np.float32)
    Wq = np.asarray(Wq, np.float32)
    Wk = np.asarray(Wk, np.float32)
    Wv = np.asarray(Wv, np.float32)
    Wo = np.asarray(Wo, np.float32)
    bq = np.asarray(bq, np.float32)
    bk = np.asarray(bk, np.float32)
    bv = np.asarray(bv, np.float32)
    bo = np.asarray(bo, np.float32)

    in_maps = []
    for c in range(NCORES):
        b, hh = c // 2, c % 2
        own = slice(hh * 512, (hh + 1) * 512)
        bqk = np.ascontiguousarray(
            np.stack([bq[own].reshape(OCH, P).T,
                      bk[own].reshape(OCH, P).T], 1))        # [128, 2, 4]
        in_maps.append({
            "xT": np.ascontiguousarray(x[b].T).astype(bf),
            "wq": np.ascontiguousarray(Wq[own, :].T).astype(bf),
            "wk": np.ascontiguousarray(Wk[own, :].T).astype(bf),
            "wv": np.ascontiguousarray(Wv[own, :].T).astype(bf),
            "wo": np.ascontiguousarray(Wo[:, own].T).astype(bf),
            "csk": csk.astype(bf),
            "r2T": r2T.astype(bf),
            "bqk": bqk,
            "bvb": np.ascontiguousarray(
                np.broadcast_to(bv[own], (P, 512))).astype(bf),
        })

    res = run_bass_kernel_spmd(nc, in_maps, list(range(NCORES)))
    LAST_EXEC_TIME_NS = res.exec_time_ns

    out = np.empty((B, S, DIM), np.float32)
    for b in range(B):
        pa = res.results[2 * b]["outT"].astype(np.float32)
        pb = res.results[2 * b + 1]["outT"].astype(np.float32)
        out[b] = (pa + pb).T + bo
    return out

